# revision 22
# baseline (speedup 1.0000x reference)
"""Trainium2 Bass kernel for nn_Jacobi_layer: 20 Jacobi sweeps over 32
independent 512x512 grids (5-point stencil, reflect padding, Dirichlet mask,
source term f = COF*layout with COF ~ 1e-8 -- numerically negligible, dropped).

Sharding: pure data parallelism -- 4 samples per core across 8 NeuronCores.
State is bf16 (rel err ~9e-3 over 20 sweeps, gate 2e-2).

Design (v2): 5-chunk layout with GHOST ROWS in the partition dimension.
Grid rows are split into 5 chunks -- (127, 126, 126, 126, 7) data rows --
each stored on partitions with one ghost partition-row on each interior
boundary (a copy of the neighbouring chunk's edge row). The vertical
(cross-partition) stencil for a chunk is then ONE tridiagonal matmul with no
cross-chunk corner fixups (the baseline needed 6 extra corner matmuls).
Ghost rows are refreshed each iteration by SBUF->SBUF DMAs (2 per block) on
the otherwise idle DMA queues.

Per block (sample s, iteration t) engine budget:
  - TensorE : 5 tridiagonal matmuls (V = 0.25*(up+down), LHS carries the
              0.25 and the reflect 0.5 rows) + 5 identity matmuls
              (psum += 0.25*T) -- 10 x N=512 streams.
  - VectorE : T = x_left + x_right for the NEXT block (one shifted
              tensor_add over the ghost-padded tile), the chunk-4 psum
              evacuation, and the ghost-column refresh of the previous
              block's output.
  - ScalarE : psum->bf16 evacuation of chunks 0..3 (two ACTIVATEs).
  - GpSimd  : Dirichlet mask (3 single-column memsets).
  - DMA     : 2 ghost-row refresh DMAs per block.

PSUM is managed as an explicit 8-bank ring (5 banks per block, stride 2560
fp32 mod 4096). Consecutive blocks overlap in 2 banks; the PE issue order
(chunks 0,1,2 + their identity matmuls first, chunks 3,4 last) gives the
previous block's ScalarE copy a ~1.5us head start on the reused banks.
"""
import sys
import numpy as np

if "/opt/trn_rl_repo" not in sys.path:
    sys.path.insert(0, "/opt/trn_rl_repo")

from contextlib import ExitStack

import ml_dtypes
import concourse.bass as bass
import concourse.bacc as bacc
import concourse.tile as tile
import concourse.mybir as mybir
from concourse.bass_utils import run_bass_kernel_spmd

NX = 512
P = 128
NCHUNK = 5
PW = NX + 4        # padded chunk width: [pad, ghostL, 512 data, ghostR, pad]
DOFF = 2           # data offset inside a chunk (even -> 4B-aligned bf16)
BATCH = 32
NCORES = 8
SPC = BATCH // NCORES   # samples per core = 4
WP = NCHUNK * PW        # 2580 padded free elems per x tile
WPA = 2816              # x tile free size (512B-multiple base stride)
WT = NCHUNK * NX        # 2560 compact free elems (T tile / psum per block)
PSUM_W = 4096           # whole PSUM: 8 banks x 512 fp32

# (data_row0, n_rows, partition0) per chunk; ghost rows sit at p0/p127.
CHUNKS = [(0, 127, 0), (127, 126, 1), (253, 126, 1), (379, 126, 1),
          (505, 7, 1)]

BF16 = mybir.dt.bfloat16
F32 = mybir.dt.float32
NP_BF16 = ml_dtypes.bfloat16


def _build_consts() -> np.ndarray:
    """[128, 520] bf16: A_top | A_mid | A_bot | 0.25*I | mask cols."""
    c = np.zeros((P, 520), dtype=np.float32)
    top = c[:, 0:128]
    for m in range(1, 127):
        top[m - 1, m] = 0.25
        top[m + 1, m] = 0.25
    top[1, 0] = 0.5           # reflect: row 0 vertical sum = 2*x[1]
    mid = c[:, 128:256]
    for m in range(1, 127):
        mid[m - 1, m] = 0.25
        mid[m + 1, m] = 0.25
    bot = c[:, 256:384]       # chunk 4: rows 505..511 at m=1..7, K=0..7
    for m in range(1, 7):
        bot[m - 1, m] = 0.25
        bot[m + 1, m] = 0.25
    bot[6, 7] = 0.5           # reflect: row 511 vertical sum = 2*x[510]
    c[:, 384:512] = 0.25 * np.eye(P, dtype=np.float32)
    # Dirichlet mask columns (zero col 0 of global rows 128..383): one
    # 0/1 column per chunk 1..3 in this chunk's partition numbering.
    # Ghost-row partitions get 0/1 arbitrarily -- the ghost-row DMA
    # overwrites them with post-mask values.
    c[:, 512:515] = 1.0
    c[2:127, 512] = 0.0   # chunk 1: rows 128..252 at p2..126
    c[0:128, 513] = 0.0   # chunk 2: rows 253..378 (all masked)
    c[0:6, 514] = 0.0     # chunk 3: rows 379..383 at p1..5
    return c.astype(NP_BF16)


def _build(n_iter: int):
    nc = bacc.Bacc("TRN2", target_bir_lowering=False, debug=False,
                   num_devices=NCORES)

    heat_d = nc.dram_tensor("heat", [SPC, NCHUNK, P, PW], BF16,
                            kind="ExternalInput")
    cst_d = nc.dram_tensor("consts", [P, 520], BF16, kind="ExternalInput")
    out_d = nc.dram_tensor("out", [SPC, NX, NX], BF16, kind="ExternalOutput")

    with tile.TileContext(nc) as tc:
        with ExitStack() as ctx:
            state = ctx.enter_context(tc.tile_pool(name="state", bufs=1))
            tpool = ctx.enter_context(tc.tile_pool(name="tpool", bufs=4))
            ppool = ctx.enter_context(
                tc.tile_pool(name="ppool", bufs=1, space=bass.MemorySpace.PSUM))

            cst = state.tile([P, 520], BF16, tag="cst")
            nc.sync.dma_start(cst[:], cst_d.ap())
            lhs_top = cst[:, 0:128]
            lhs_mid = cst[:, 128:256]
            lhs_bot = cst[0:8, 256:384]
            lhs_idn = cst[:, 384:512]
            maskc = cst[:, 512:515].rearrange("p (c j) -> p c j", c=3)

            ring = ppool.tile([P, PSUM_W], F32, tag="ring", name="ring")

            xa, xb = [], []
            dma_eng = [nc.sync, nc.scalar, nc.gpsimd, nc.sync]
            for s in range(SPC):
                x0 = state.tile([P, WPA], BF16, tag=f"xa{s}", name=f"xa{s}")
                # different engines -> different DMA queues -> parallel loads
                dma_eng[s].dma_start(
                    x0[:, 0:WP].rearrange("p (c j) -> p c j", c=NCHUNK),
                    heat_d.ap()[s].rearrange("c p j -> p c j"))
                xa.append(x0)
                xb.append(state.tile([P, WPA], BF16, tag=f"xb{s}",
                                     name=f"xb{s}"))

            def x3(x):
                return x[:, 0:WP].rearrange("p (c j) -> p c j", c=NCHUNK)

            def make_T(xsrc):
                """DVE: T = x_left + x_right over the ghost-padded tile."""
                T = tpool.tile([P, WT], BF16, tag="T", name="T")
                t3 = T.rearrange("p (c j) -> p c j", c=NCHUNK)
                xs = x3(xsrc)
                nc.vector.tensor_add(
                    t3[:, :, 0:NX], xs[:, :, 1:NX + 1], xs[:, :, 3:NX + 3])
                return T

            # PE warm-up on the (small, fast-arriving) consts tile covers
            # the parallel heat DMAs so the PE ramps to 2.4 GHz first.
            for _k in range(22):
                nc.tensor.matmul(ring[:, 0:NX], lhs_idn, cst[:, 0:NX],
                                 start=True, stop=True)

            def emit_copies(xnew, offs, chunks, eng):
                """Evacuate psum chunk regions -> x_new data cols, merging
                chunks whose ring regions are contiguous."""
                xn3 = x3(xnew)
                i = 0
                while i < len(chunks):
                    j = i
                    while (j + 1 < len(chunks)
                           and chunks[j + 1] == chunks[j] + 1
                           and offs[chunks[j + 1]] == offs[chunks[j]] + NX):
                        j += 1
                    c0, n = chunks[i], j - i + 1
                    src = ring[:, offs[c0]:offs[c0] + n * NX].rearrange(
                        "p (c j) -> p c j", c=n)
                    dst = xn3[:, c0:c0 + n, DOFF:DOFF + NX]
                    if eng == "scalar":
                        nc.scalar.copy(dst, src)
                    else:
                        nc.vector.tensor_copy(dst, src)
                    i = j + 1

            cur, nxt = xa, xb
            nblocks = n_iter * SPC
            # T pipeline runs TWO blocks ahead of the PE so the identity
            # matmuls never wait on the DVE.
            T_queue = [make_T(xa[0]), make_T(xa[1])]
            pending = None            # x_new awaiting ghost refresh
            for t in range(n_iter):
                last_t = t == n_iter - 1
                for s in range(SPC):
                    b = t * SPC + s
                    base = (b * WT) % PSUM_W
                    offs = [(base + NX * c) % PSUM_W for c in range(NCHUNK)]
                    x, xn = cur[s], nxt[s]
                    xv = x3(x)

                    # --- finalize previous block: ghost-row DMAs. They copy
                    # DATA cols only -- ghost rows never need ghost cols
                    # (their T values land in unused psum slots) -- so the
                    # DMAs wait only on the copies + mask, not on the
                    # GpSimd ghost-col ops.
                    if pending is not None:
                        dn_dst = pending[127:128, 0:4 * PW].rearrange(
                            "p (c j) -> p c j", c=4)
                        dn_src = pending[1:2, PW:WP].rearrange(
                            "p (c j) -> p c j", c=4)
                        up_dst = pending[0:1, PW:WP].rearrange(
                            "p (c j) -> p c j", c=4)
                        up_src = pending[126:127, 0:4 * PW].rearrange(
                            "p (c j) -> p c j", c=4)
                        # down ghosts: chunk c p127 <- chunk c+1 p1
                        nc.sync.dma_start(dn_dst[:, :, DOFF:DOFF + NX],
                                          dn_src[:, :, DOFF:DOFF + NX])
                        # up ghosts: chunk c+1 p0 <- chunk c p126
                        nc.sync.dma_start(up_dst[:, :, DOFF:DOFF + NX],
                                          up_src[:, :, DOFF:DOFF + NX])
                        pending = None

                    # --- DVE: T for block b+2 (2-block pipeline), emitted
                    # FIRST so its semaphore waits bind to already-finished
                    # work.
                    nb = b + 2
                    if nb < nblocks:
                        t2, s2 = divmod(nb, SPC)
                        T_queue.append(
                            make_T(cur[s2] if t2 == t else nxt[s2]))

                    def tri(c, lhs, start, kp=P, nchunk=1):
                        nc.tensor.matmul(
                            ring[:, offs[c]:offs[c] + nchunk * NX], lhs,
                            xv[0:kp, c:c + nchunk, DOFF:DOFF + NX],
                            start=start, stop=not start)

                    prev_T = T_queue.pop(0)

                    def idn(c, start, nchunk=1):
                        nc.tensor.matmul(
                            ring[:, offs[c]:offs[c] + nchunk * NX], lhs_idn,
                            prev_T[:, c * NX:(c + nchunk) * NX],
                            start=start, stop=not start)

                    # Matmul issue order: chunks 0..2 finish in the first
                    # 4 matmuls and each psum region's evacuation is
                    # EMITTED right after the matmul that completes it, so
                    # every cross-engine wait binds to work that is already
                    # done and the evacuations start as early as possible.
                    # tri1+tri2 / idn0+idn1 merge to N=1024 when their ring
                    # regions don't straddle the wrap.
                    tri(0, lhs_top, True)
                    tri(1, lhs_mid, True)
                    tri(2, lhs_mid, True)
                    idn(0, False)
                    idn(1, False)
                    emit_copies(xn, offs, [0, 1], "scalar")
                    idn(2, False)
                    emit_copies(xn, offs, [2], "scalar")
                    tri(3, lhs_mid, True)
                    tri(4, lhs_bot, True, kp=8)
                    idn(3, False)
                    emit_copies(xn, offs, [3], "scalar")
                    idn(4, False)
                    emit_copies(xn, offs, [4], "vector")

                    # --- GpSimd: Dirichlet mask (zero col 0, rows 128..383)
                    # via a per-partition 0/1 column product (compute APs
                    # must start at partition 0/32/64/96), then the ghost
                    # column refresh (feeds T-builds ~2 blocks later).
                    xn3m = x3(xn)
                    nc.gpsimd.tensor_mul(
                        xn3m[:, 1:4, DOFF:DOFF + 1],
                        xn3m[:, 1:4, DOFF:DOFF + 1], maskc)
                    if not last_t:
                        nc.gpsimd.tensor_copy(
                            xn3m[:, :, 1:2], xn3m[:, :, 3:4])
                        nc.gpsimd.tensor_copy(
                            xn3m[:, :, PW - 2:PW - 1],
                            xn3m[:, :, PW - 4:PW - 3])

                    if not last_t:
                        pending = xn
                cur, nxt = nxt, cur

            out_eng = [nc.sync, nc.scalar, nc.gpsimd, nc.sync]
            for s in range(SPC):
                for c, (r0, nr, p0) in enumerate(CHUNKS):
                    out_eng[(s + c) % 4].dma_start(
                        out_d.ap()[s][r0:r0 + nr, :],
                        cur[s][p0:p0 + nr,
                               c * PW + DOFF:c * PW + DOFF + NX])

    nc.compile()
    return nc


_CACHE: dict = {}


def _get_nc(n_iter: int):
    if n_iter not in _CACHE:
        _CACHE[n_iter] = _build(n_iter)
    return _CACHE[n_iter]


def _prep_heat(heat: np.ndarray) -> np.ndarray:
    """[B,512,512] fp32 -> [B,5,128,516] bf16 5-chunk layout with ghost
    partition-rows, ghost cols, and the Dirichlet mask applied."""
    b = heat.shape[0]
    h = heat.copy()
    h[:, 128:384, 0] = 0.0    # x0 = heat * G
    hp = np.zeros((b, NCHUNK, P, PW), dtype=np.float32)
    for c, (r0, nr, p0) in enumerate(CHUNKS):
        hp[:, c, p0:p0 + nr, DOFF:DOFF + NX] = h[:, r0:r0 + nr, :]
        if c == 0:
            hp[:, c, 127, DOFF:DOFF + NX] = h[:, 127, :]
        else:
            hp[:, c, 0, DOFF:DOFF + NX] = h[:, r0 - 1, :]
            if c < 4:
                hp[:, c, 127, DOFF:DOFF + NX] = h[:, r0 + nr, :]
    hp[..., DOFF - 1] = hp[..., DOFF + 1]          # ghost-left = col 1
    hp[..., DOFF + NX] = hp[..., DOFF + NX - 2]    # ghost-right = col 510
    return hp.astype(NP_BF16)


def run(layout, heat, n_iter, trace=False):
    n_iter = int(n_iter)
    heat = np.ascontiguousarray(np.asarray(heat, dtype=np.float32)
                                .reshape(BATCH, NX, NX))
    hp = _prep_heat(heat)
    consts = _build_consts()
    nc = _get_nc(n_iter)
    in_maps = []
    for c in range(NCORES):
        sl = slice(c * SPC, (c + 1) * SPC)
        in_maps.append({"heat": hp[sl], "consts": consts})
    res = run_bass_kernel_spmd(nc, in_maps, list(range(NCORES)), trace=trace)
    out = np.concatenate(
        [res.results[c]["out"].reshape(SPC, NX, NX) for c in range(NCORES)],
        axis=0)
    return out.astype(np.float32).reshape(BATCH, 1, NX, NX), res


def kernel(layout, heat, n_iter):
    out, _ = run(layout, heat, n_iter)
    return out
